# revision 61
# baseline (speedup 1.0000x reference)
"""Causal multi-head attention (B=4, T=2048, C=1024, H=16) on 8 Trainium2 cores.

Sharding: core c handles batch b = c//2 and heads h0..h0+7 with h0 = (c%2)*8.
Each core computes QKV projection for its head slice, causal attention for its
8 heads, and a partial output projection. Host sums the two partials per batch
and adds the bias terms.

All tensors are bf16 on the PE (psum accumulation stays f32); rel err ~4e-3.

Attention works in the S^T = K Q^T layout ([k, q], k on partitions) so that
softmax needs no cross-partition reduction. The AV matmuls run in [q, d]
orientation (stationary = exp'd score block, moving = 65-wide v head slot
whose last column is all-ones, emitting the softmax row-sums for free), so
normalization is a per-partition reciprocal + scalar multiply and AV costs
~2x fewer PE cycles than the [d, q] orientation. Each (q-subtile, head) AV
chain runs chain-major (PSUM accumulation groups must not interleave within
a bank). Normalized head pairs are re-transposed to [c, t] with a PE
identity-transpose for the projection.

The attention strips over queries are Act-engine-bound (exp) while the QKV
projection is pure PE work, so QKV for t-strip s is deferred until just
before the attention strip that first needs it and drained through a
pending-work queue between the score matmuls — the in-order PE then always
has independent matmuls to run while the Act engine catches up.
"""

import os
import sys
import numpy as np

sys.path.insert(0, "/opt/trn_rl_repo")

import concourse.bass as bass  # noqa: E402
import concourse.bacc as bacc  # noqa: E402
import concourse.mybir as mybir  # noqa: E402
from concourse.bass_utils import run_bass_kernel_spmd  # noqa: E402
from concourse.tile import TileContext  # noqa: E402

B, T, C, H = 4, 2048, 1024, 16
HD = C // H          # 64 head dim
HPC = 8              # heads per core
P = 128
NT = T // P          # 16 t-chunks of 128
NS = T // 512        # 4 q-strips of 512
KC = C // P          # 8 contraction chunks for QKV
CL = HPC * HD        # 512 local channels per section
F32 = mybir.dt.float32
F32R = mybir.dt.float32r
BF16 = mybir.dt.bfloat16
EXPF = mybir.ActivationFunctionType.Exp
MUL = mybir.AluOpType.mult

_CACHED = {}
_MARKS = []


def build_nc():
    nc = bacc.Bacc("TRN2", target_bir_lowering=False, debug=False)

    xt_d = nc.dram_tensor("xt", [C, T], BF16, kind="ExternalInput")
    wqk_d = nc.dram_tensor("wqk", [C, 2 * CL], BF16, kind="ExternalInput")
    wv_d = nc.dram_tensor("wv", [C, CL], BF16, kind="ExternalInput")
    wp_d = nc.dram_tensor("wp", [CL, C], BF16, kind="ExternalInput")
    bqk_d = nc.dram_tensor("bqk", [P, 8], F32, kind="ExternalInput")
    ident_d = nc.dram_tensor("ident", [P, P], BF16, kind="ExternalInput")
    maskb_d = nc.dram_tensor("maskb", [P, P], BF16, kind="ExternalInput")
    ones_d = nc.dram_tensor("ones", [P, NT * HPC], BF16, kind="ExternalInput")
    y_d = nc.dram_tensor("y", [T, C], F32, kind="ExternalOutput")
    dbg_d = (nc.dram_tensor("dbg", [P, 4, T], BF16, kind="ExternalOutput")
             if os.environ.get("KDBG") else None)

    xt_r = xt_d.ap().rearrange("(kc p) t -> p kc t", p=P)       # [128, 8, 2048]
    wqk_r = wqk_d.ap().rearrange("(kc p) c -> p kc c", p=P)     # [128, 8, 1024]
    wv_r = wv_d.ap().rearrange("(kc p) c -> p kc c", p=P)       # [128, 8, 512]
    wp_r = wp_d.ap().rearrange("(ct p) c -> p ct c", p=P)       # [128, 4, 1024]
    y_r = y_d.ap().rearrange("(tt p) c -> p tt c", p=P)         # [128, 16, 1024]

    SW = 256            # QKV t-strip width (NSA = 8 strips)
    NSA = T // SW
    EH = HD + 1         # 65: head slot width in v (value cols + ones col)
    scale = float(HD) ** -0.5

    with TileContext(nc) as tc:
      with (
          tc.tile_pool(name="const", bufs=1) as constp,
          tc.tile_pool(name="big", bufs=1) as bigp,
          tc.tile_pool(name="xts", bufs=4) as xtsp,
          tc.tile_pool(name="u_pool", bufs=30) as up,
          tc.tile_pool(name="apair_p", bufs=4) as apairp,
          tc.tile_pool(name="rc_p", bufs=4) as rcp,
          tc.tile_pool(name="ystage", bufs=6) as ystagep,
          tc.tile_pool(name="ps2", bufs=2, space="PSUM") as ps2p,
          tc.tile_pool(name="ps_av", bufs=2, space="PSUM") as psavp,
          tc.tile_pool(name="ps_pp", bufs=2, space="PSUM") as psppp,
      ):
        v_sb = bigp.tile([P, NT, HPC * EH], BF16)
        v_heads = v_sb[:].rearrange("p t (h e) -> p t h e", e=EH)
        qkT = bigp.tile([P, 8, T], BF16)  # c-tiles 0-3 = qT, 4-7 = kT
        attnT = bigp.tile([P, 4, T], BF16)
        wp_sb = bigp.tile([P, 4, C], BF16)
        wv_sb = bigp.tile([P, KC, CL], BF16)
        wqk_sb = bigp.tile([P, KC, 2 * CL], BF16)

        # ---- initial DMAs (interleaved so the first v chain starts early,
        # wqk streams during the early v work) ----
        xts_l = [None] * NSA
        xts_l[0] = xtsp.tile([P, KC, SW], BF16, tag="xts", name="xts0")
        nc.sync.dma_start(xts_l[0][:, 0:1, :], xt_r[:, 0:1, 0:SW])
        nc.scalar.dma_start(wv_sb[:, 0:1, :], wv_r[:, 0:1, :])
        nc.sync.dma_start(xts_l[0][:, 1:4, :], xt_r[:, 1:4, 0:SW])
        nc.sync.dma_start(wv_sb[:, 1:4, :], wv_r[:, 1:4, :])
        nc.sync.dma_start(xts_l[0][:, 4:8, :], xt_r[:, 4:8, 0:SW])
        nc.sync.dma_start(wv_sb[:, 4:8, :], wv_r[:, 4:8, :])
        xts_l[1] = xtsp.tile([P, KC, SW], BF16, tag="xts", name="xts1")
        nc.sync.dma_start(xts_l[1][:], xt_r[:, :, SW:2 * SW])
        bqk = constp.tile([P, 8], F32)
        nc.sync.dma_start(bqk[:], bqk_d[:])
        for qc in range(4):  # wqk in chunks so qk work can start early
            nc.sync.dma_start(
                wqk_sb[:, :, qc * 256:(qc + 1) * 256],
                wqk_r[:, :, qc * 256:(qc + 1) * 256])
        ident = constp.tile([P, P], BF16)
        nc.sync.dma_start(ident[:], ident_d[:])
        maskb = constp.tile([P, P], BF16)
        nc.sync.dma_start(maskb[:], maskb_d[:])
        nc.sync.dma_start(v_heads[:, :, :, HD], ones_d[:])
        nc.sync.dma_start(wp_sb[:], wp_r)

        # ---- QKV work units (PE chains through the shared pp psum ring) ----
        def v_work_tt(ts, tt):
            def run():
                xts = xts_l[ts]
                tch = ts * (SW // P) + tt
                psv = psppp.tile([P, CL], F32, tag="pp", name=f"psv_{tch}")
                for kc in range(KC):
                    nc.tensor.matmul(
                        psv[:],
                        xts[:, kc, tt * P:(tt + 1) * P],
                        wv_sb[:, kc, :],
                        start=(kc == 0), stop=(kc == KC - 1),
                    )
                nc.vector.tensor_copy(
                    v_heads[:, tch, :, 0:HD],
                    psv[:].rearrange("p (h d) -> p h d", d=HD),
                )
            return run

        def qk_work_ct(ts, ct):
            def run():
                xts = xts_l[ts]
                psq = psppp.tile([P, SW], F32, tag="pp",
                                 name=f"psq_{ts}_{ct}")
                for kc in range(KC):
                    nc.tensor.matmul(
                        psq[:],
                        wqk_sb[:, kc, ct * P:(ct + 1) * P],
                        xts[:, kc, :],
                        start=(kc == 0), stop=(kc == KC - 1),
                    )
                nc.vector.tensor_scalar_add(
                    qkT[:, ct, ts * SW:(ts + 1) * SW],
                    psq[:],
                    bqk[:, ct:ct + 1],
                )
            return run

        # ---- pending-work queues (labelled closures) ----
        # qA: QKV-strip work + transposes, drained eagerly. qP: projection
        # chains, drained only when qA is empty and the current strip is
        # late enough -- the last attention strip is Act-bound, so the
        # projections of strips 0..2 are saved to fill its PE idle time.
        qA = []  # (label, closure); label ("A", ts) for QKV strip work
        qP = []
        a_left = [0] * NSA
        drain_ctl = {"pmax": 0}

        def drain(n):
            for _ in range(n):
                if qA:
                    lbl, fn = qA.pop(0)
                    if lbl is not None:
                        a_left[lbl[1]] -= 1
                    fn()
                elif qP and drain_ctl["pmax"] > 0:
                    drain_ctl["pmax"] -= 1
                    qP.pop(0)()
                else:
                    return

        def push_astrip(ts):
            xts_l[ts] = xtsp.tile([P, KC, SW], BF16, tag="xts",
                                  name=f"xts{ts}")
            nc.sync.dma_start(xts_l[ts][:],
                              xt_r[:, :, ts * SW:(ts + 1) * SW])
            items = ([v_work_tt(ts, tt) for tt in range(SW // P)]
                     + [qk_work_ct(ts, ct) for ct in range(8)])
            a_left[ts] = len(items)
            for fn in items:
                qA.append((("A", ts), fn))

        def flush_astrip(ts):
            while a_left[ts] > 0:
                drain(1)

        def proj_chain(tt, co):
            def run():
                if drain_ctl.get("alt") and (tt + co) % 2:
                    psy = ps2p.tile([P, 512], F32, tag="ps2",
                                    name=f"psy_{tt}_{co}")
                else:
                    psy = psppp.tile([P, 512], F32, tag="pp",
                                     name=f"psy_{tt}_{co}")
                for ct in range(4):
                    nc.tensor.matmul(
                        psy[:],
                        attnT[:, ct, tt * P:(tt + 1) * P],
                        wp_sb[:, ct, co * 512:(co + 1) * 512],
                        start=(ct == 0), stop=(ct == 3),
                    )
                yt = ystagep.tile([P, 512], F32, tag="yt",
                                  name=f"yt_{tt}_{co}")
                if drain_ctl.get("alt") and (tt + co) % 2:
                    nc.scalar.copy(yt[:], psy[:])
                else:
                    nc.vector.tensor_copy(yt[:], psy[:])
                nc.sync.dma_start(
                    y_r[:, tt, co * 512:(co + 1) * 512], yt[:])
            return run

        def transp(pr, qj, qsub, apair):
            def run():
                psT = psppp.tile([P, P], BF16, tag="pp",
                                 name=f"psT_{qj}_{pr}_{qsub}")
                nc.tensor.transpose(psT[:], apair[:, qsub, :], ident[:])
                t0 = qj * 512 + qsub * P
                nc.vector.tensor_copy(attnT[:, pr, t0:t0 + P], psT[:])
            return run

        # ---- preamble: QKV for t < 512 (needed by attention strip 0) ----
        # preload the Exp activation table while the Act engine is idle so
        # the first real exp doesn't pay the table-load latency
        warm = constp.tile([P, 8], F32, tag="warm")
        nc.scalar.activation(warm[:], bqk[:], EXPF)
        for ts in (0, 1):
            for tt in range(SW // P):
                v_work_tt(ts, tt)()
        for ts in (0, 1):
            for ct in range(8):
                qk_work_ct(ts, ct)()

        # ---- attention strips (QKV for later t deferred via the queue) ----
        for qj in range(NS):
            s0 = 2 * (qj + 1)
            for s in (s0, s0 + 1):
                if s < NSA:
                    push_astrip(s)
            # the strips feeding THIS attention strip must be emitted
            if qj > 0:
                flush_astrip(2 * qj)
                flush_astrip(2 * qj + 1)
            drain_ctl["pmax"] = {0: 0, 1: 0, 2: 0, 3: 10 ** 9}[qj]
            _MARKS.append((f"strip{qj}", nc.next_id()))
            nk = 4 * (qj + 1)
            pend_tail = [None]
            for pr in range(4):  # head pair (2pr, 2pr+1)
                qct, kct = pr, 4 + pr
                st3 = True
                if not st3:
                    avA = psavp.tile([P, 4, P], F32, tag="av",
                                     name=f"avA_{qj}_{pr}")
                    avB = psavp.tile([P, 4, P], F32, tag="av",
                                     name=f"avB_{qj}_{pr}")
                    apair = apairp.tile([P, 4, P], BF16, tag="ap",
                                        name=f"apair_{qj}_{pr}")
                # PSUM accumulation groups must not interleave within a
                # bank, so each (qsub, head) AV chain runs chain-major,
                # lagging the exp of its last (diagonal) k-tile by 2
                # steps. u tiles live for the whole pair.
                u_list = [None] * nk
                box = {}
                if not st3:
                    box["avA"], box["avB"], box["ap"] = avA, avB, apair

                def av_chain(qsub, u_list=u_list, box=box, qj=qj, pr=pr):
                    kend = 4 * qj + qsub
                    for hh in range(2):
                        av = box["avA"] if hh == 0 else box["avB"]
                        for kt in range(kend + 1):
                            nc.tensor.matmul(
                                av[:, qsub, 0:EH],
                                u_list[kt][:, hh,
                                           qsub * P:(qsub + 1) * P],
                                v_heads[:, kt, 2 * pr + hh, :],
                                start=(kt == 0),
                                stop=(kt == kend),
                            )

                ps_pend = []

                def emit_scores(kt, ps_pend=ps_pend, u_list=u_list,
                                qj=qj, pr=pr, qct=None, kct=None):
                    qct, kct = pr, 4 + pr
                    q0 = max(0, kt * P - qj * 512)
                    diag = kt >= 4 * qj
                    ps = ps2p.tile([P, 2, 512], F32, tag="ps2",
                                   name=f"ps_{qj}_{pr}_{kt}")
                    u = up.tile([P, 2, 512], BF16, tag="u",
                                name=f"u_{qj}_{pr}_{kt}")
                    u_list[kt] = u
                    for hh in range(2):
                        hp = hh * HD
                        nc.tensor.matmul(
                            ps[:, hh, q0:512],
                            qkT[hp:hp + HD, kct,
                                kt * P:(kt + 1) * P],
                            qkT[hp:hp + HD, qct,
                                qj * 512 + q0:(qj + 1) * 512],
                            start=True, stop=not diag,
                        )
                    if diag:
                        for hh in range(2):
                            nc.tensor.matmul(
                                ps[:, hh, q0:q0 + P],
                                ident[:], maskb[:],
                                start=False, stop=True,
                            )
                    ps_pend.append((kt, ps, u, q0))

                def emit_exps(ps_pend=ps_pend):
                    while ps_pend:
                        kt, ps, u, q0 = ps_pend.pop(0)
                        nc.scalar.activation(
                            u[:, :, q0:512], ps[:, :, q0:512],
                            EXPF, scale=scale,
                        )

                def step_post(step, qj=qj, pr=pr, nk=nk,
                              av_chain=av_chain):
                    rem = (nk + 2 - step) + (3 - pr) * (nk + 2)
                    backlog = len(qA) + (len(qP) if drain_ctl["pmax"] > 0
                                         else 0)
                    if backlog >= rem:
                        drain(2)
                    elif 2 * backlog >= rem or 2 * step >= nk:
                        drain(1)
                    if 4 * qj + 2 <= step:
                        qsub = step - 4 * qj - 2
                        if 0 <= qsub < 4:
                            # v strips feeding this chain must be emitted
                            flush_astrip((4 * qj + qsub) // 2)
                            av_chain(qsub)

                def tail(box=box, qj=qj, pr=pr, step_post=step_post):
                    step_post(nk)
                    step_post(nk + 1)
                    _MARKS.append((f"s{qj}p{pr}_tail", nc.next_id()))
                    avA, avB = box["avA"], box["avB"]
                    apair = box["ap"]
                    rc = rcp.tile([P, 2, 4], F32, tag="rc",
                                  name=f"rc_{qj}_{pr}")
                    nc.vector.reciprocal(rc[:, 0, :], avA[:, :, HD])
                    nc.vector.reciprocal(rc[:, 1, :], avB[:, :, HD])
                    for qsub in range(4):
                        for hh in range(2):
                            av = avA if hh == 0 else avB
                            nc.vector.tensor_scalar_mul(
                                apair[:, qsub, hh * HD:(hh + 1) * HD],
                                av[:, qsub, 0:HD],
                                rc[:, hh, qsub:qsub + 1],
                            )
                    for qsub in range(4):
                        qA.append((None, transp(pr, qj, qsub, apair)))

                if st3:
                    # pipeline: emit this pair's first score/exp group
                    # before the previous pair's tail so the Act engine
                    # never drains at the pair boundary
                    emit_scores(0)
                    emit_scores(1)
                    emit_exps()
                    if pend_tail[0] is not None:
                        pend_tail[0]()
                    box["avA"] = psavp.tile([P, 4, P], F32, tag="av",
                                            name=f"avA_{qj}_{pr}")
                    box["avB"] = psavp.tile([P, 4, P], F32, tag="av",
                                            name=f"avB_{qj}_{pr}")
                    box["ap"] = apairp.tile([P, 4, P], BF16, tag="ap",
                                            name=f"apair_{qj}_{pr}")
                    for kt2 in range(2, nk, 2):
                        emit_scores(kt2)
                        emit_scores(kt2 + 1)
                        emit_exps()
                        step_post(kt2)
                        step_post(kt2 + 1)
                    pend_tail[0] = tail
                else:
                    for kt2 in range(0, nk, 2):
                        emit_scores(kt2)
                        emit_scores(kt2 + 1)
                        emit_exps()
                        step_post(kt2)
                        step_post(kt2 + 1)
                    tail()
            if pend_tail[0] is not None:
                pend_tail[0]()
            # strip end: queue this strip's projection chains (deferred)
            for t4 in range(4):
                for co in range(2):
                    qP.append(proj_chain(4 * qj + t4, co))
        _MARKS.append(("final", nc.next_id()))
        drain_ctl["alt"] = True
        while qA or qP:
            drain_ctl["pmax"] = 10 ** 9
            drain(1)
        if dbg_d is not None:
            nc.sync.dma_start(dbg_d.ap(), attnT[:])
    nc.compile()
    return nc


def _host_consts():
    import ml_dtypes
    i_idx = np.arange(P, dtype=np.float32)[:, None]
    j_idx = np.arange(P, dtype=np.float32)[None, :]
    maskb = np.where(j_idx - i_idx >= 0, 0.0, -1e30).astype(ml_dtypes.bfloat16)
    ident = np.eye(P, dtype=ml_dtypes.bfloat16)
    ones = np.ones((P, NT * HPC), dtype=ml_dtypes.bfloat16)
    return ident, maskb, ones


def make_in_maps(x, w_attn, b_attn, w_proj):
    import ml_dtypes
    bf16 = ml_dtypes.bfloat16
    ident, maskb, ones = _host_consts()
    in_maps = []
    for c in range(8):
        b = c // 2
        h0 = (c % 2) * HPC
        qcols = slice(h0 * HD, h0 * HD + CL)
        kcols = slice(C + h0 * HD, C + h0 * HD + CL)
        vcols = slice(2 * C + h0 * HD, 2 * C + h0 * HD + CL)
        wqk = np.concatenate([w_attn[:, qcols], w_attn[:, kcols]], axis=1)
        bqk = np.concatenate([b_attn[qcols], b_attn[kcols]]).reshape(8, P).T
        in_maps.append({
            "xt": np.ascontiguousarray(x[b].T).astype(bf16),
            "wqk": np.ascontiguousarray(wqk).astype(bf16),
            "wv": np.ascontiguousarray(w_attn[:, vcols]).astype(bf16),
            "wp": np.ascontiguousarray(
                w_proj[h0 * HD:h0 * HD + CL, :]).astype(bf16),
            "bqk": np.ascontiguousarray(bqk),
            "ident": ident,
            "maskb": maskb,
            "ones": ones,
        })
    return in_maps


def _get_runner():
    """Build the SPMD executor once: a cached jax.jit over 8 cores.

    Mirrors bass2jax.run_bass_via_pjrt but hoists the jit so repeated
    kernel() calls reuse the compiled executable.
    """
    if "runner" in _CACHED:
        return _CACHED["runner"]
    import jax
    import jax.numpy as jnp
    from jax.sharding import Mesh, PartitionSpec
    from jax.experimental.shard_map import shard_map
    from concourse import bass2jax
    import concourse.mybir as mybir_

    nc = _CACHED.get("nc")
    if nc is None:
        nc = _CACHED["nc"] = build_nc()
    bass2jax.install_neuronx_cc_hook()

    partition_name = (nc.partition_id_tensor.name
                      if nc.partition_id_tensor else None)
    in_names, out_names, out_avals, zero_shapes = [], [], [], []
    for alloc in nc.m.functions[0].allocations:
        if not isinstance(alloc, mybir_.MemoryLocationSet):
            continue
        name = alloc.memorylocations[0].name
        if alloc.kind == "ExternalInput":
            if name != partition_name:
                in_names.append(name)
        elif alloc.kind == "ExternalOutput":
            shape = tuple(alloc.tensor_shape)
            dtype = mybir_.dt.np(alloc.dtype)
            out_names.append(name)
            out_avals.append(jax.core.ShapedArray(shape, dtype))
            zero_shapes.append((shape, dtype))
    n_params = len(in_names)
    n_outs = len(out_names)
    all_names = in_names + out_names
    if partition_name is not None:
        all_names = all_names + [partition_name]

    def _body(*args):
        operands = list(args)
        if partition_name is not None:
            operands.append(bass2jax.partition_id_tensor())
        outs = bass2jax._bass_exec_p.bind(
            *operands,
            out_avals=tuple(out_avals),
            in_names=tuple(all_names),
            out_names=tuple(out_names),
            lowering_input_output_aliases=(),
            sim_require_finite=True,
            sim_require_nnan=True,
            nc=nc,
        )
        return tuple(outs)

    devices = jax.devices()[:8]
    mesh = Mesh(np.asarray(devices), ("core",))
    in_specs = (PartitionSpec("core"),) * (n_params + n_outs)
    out_specs = (PartitionSpec("core"),) * n_outs
    donate = tuple(range(n_params, n_params + n_outs))
    sharded = jax.jit(
        shard_map(_body, mesh=mesh, in_specs=in_specs, out_specs=out_specs,
                  check_rep=False),
        donate_argnums=donate, keep_unused=True,
    )

    def run(in_maps):
        concat_in = [
            np.concatenate([np.asarray(in_maps[c][nm]) for c in range(8)],
                           axis=0)
            for nm in in_names
        ]
        concat_zeros = [
            np.zeros((8 * s[0], *s[1:]), dt) for (s, dt) in zero_shapes
        ]
        out_arrs = sharded(*concat_in, *concat_zeros)
        return [
            {nm: np.asarray(out_arrs[i]).reshape(8, *out_avals[i].shape)[c]
             for i, nm in enumerate(out_names)}
            for c in range(8)
        ]

    _CACHED["runner"] = run
    return run


def kernel(x, w_attn, b_attn, w_proj, b_proj):
    x = np.asarray(x, dtype=np.float32)
    w_attn = np.asarray(w_attn, dtype=np.float32)
    b_attn = np.asarray(b_attn, dtype=np.float32)
    w_proj = np.asarray(w_proj, dtype=np.float32)
    b_proj = np.asarray(b_proj, dtype=np.float32)

    in_maps = make_in_maps(x, w_attn, b_attn, w_proj)
    try:
        run = _get_runner()
        results = run(in_maps)
    except Exception:
        # fallback: the stock SPMD runner (slower per call, same result)
        if "nc" not in _CACHED:
            _CACHED["nc"] = build_nc()
        res = run_bass_kernel_spmd(
            _CACHED["nc"], in_maps, core_ids=list(range(8)))
        results = res.results

    # v-bias contribution: probs rows sum to 1, so attn += 1 * b_v^T, and
    # (1 b_v^T) @ w_proj = row vector b_v @ w_proj added to every position.
    extra = b_attn[2 * C:] @ w_proj + b_proj  # [C]
    out = np.empty((B, T, C), dtype=np.float32)
    for b in range(B):
        out[b] = results[2 * b]["y"] + results[2 * b + 1]["y"] + extra
    return out
